# revision 43
# baseline (speedup 1.0000x reference)
"""Trainium2 Bass kernel for nn_AttentionBlockE3 (segment-softmax GNN attention).

Strategy: host sorts edges by destination node and partitions NODES across the
8 cores (1250 nodes each) so each core owns all edges of its nodes — no
collectives needed. Edges are packed per (core, node-chunk-of-128) into a
fixed budget of T_fix 128-edge tiles (padding edges get cutoff=0 / dst=-5 so
they contribute nothing).

v2: everything fp16 (halves the HBM traffic, which is the bottleneck), and
the per-head q.k reduction is done on the Tensor engine instead of DVE:
q and k are packed feature-transposed ([4*128 feature rows, E]); DVE forms
prod = q*k in one packed fp16 multiply (2x mode), then 4 matmuls against a
constant 0/1 head-membership matrix reduce features -> logits [128 edges, 8]
directly in PSUM. v is packed d-major (col = d*8+h) so the w*v broadcast
multiply keeps the last dim packed (DVE 2x mode); the host un-permutes the
output columns at the end.

Device program per core (all static addressing, shared by all 8 cores):
  pass 1  stream qT/kT spans, prod = qT*kT (DVE 2x), 4 membership matmuls
          -> logits psum [128,8] per tile, scale by cutoff (tensor_scalar)
  chunk   global-max-subtract softmax: exp(w - C) on ACT, C = chunk max
  pass 2  stream v tiles, rhs = wexp*v (DVE 2x d-major), one-hot via
          tensor_scalar(is_equal), matmuls accumulate [128 nodes, 480] + the
          denominator [128, 8] in PSUM
  epilog  out = pso * recip(psd + 1e-30), DMA fp16 to the node slice
"""
import numpy as np

E, D, N, H = 200000, 480, 10000, 8
P = 128
NCORES = 8
SCALE = 1.0 / np.sqrt(60.0)
FG = 4           # feature groups of 128 partitions (480 padded to 512)
SPAN = 11        # edge tiles per DMA/prod-multiply span


def _perm_dmajor():
    # packed col d*8+h  <-  fused col (per-irrep head-major layout)
    perm = np.zeros(480, np.int64)
    for h in range(8):
        for d in range(60):
            if d < 16:
                fused = h * 16 + d
            elif d < 40:
                fused = 128 + h * 24 + (d - 16)
            else:
                fused = 320 + h * 20 + (d - 40)
            perm[d * 8 + h] = fused
    return perm


PERM = _perm_dmajor()


GW = 120         # feature-group width (480 = 4 * 120, no padding)


def _memb():
    # [480, 8] 0/1 head membership of each fused feature row
    m = np.zeros((480, 8), np.float16)
    for f in range(480):
        if f < 128:
            h = f // 16
        elif f < 320:
            h = (f - 128) // 24
        else:
            h = (f - 320) // 20
        m[f, h] = 1.0
    return m


def _plan_shard(dst):
    npc = N // NCORES                       # 1250 nodes per core
    CHUNKS = (npc + P - 1) // P             # 10 windows of <=128 nodes
    order = np.argsort(dst, kind="stable")
    dst_s = dst[order]
    lo = np.array([core * npc + c * P
                   for core in range(NCORES) for c in range(CHUNKS)])
    hi = np.array([core * npc + min((c + 1) * P, npc)
                   for core in range(NCORES) for c in range(CHUNKS)])
    starts = np.searchsorted(dst_s, lo, side="left")
    ends = np.searchsorted(dst_s, hi, side="left")
    counts = ends - starts
    T_fix = int(np.max((counts + P - 1) // P))
    budget = T_fix * P
    gi = np.full((NCORES, CHUNKS, budget), -1, np.int64)
    for wi in range(NCORES * CHUNKS):
        core, c = wi // CHUNKS, wi % CHUNKS
        gi[core, c, :counts[wi]] = order[starts[wi]:ends[wi]]
    return gi.reshape(NCORES, -1), T_fix, CHUNKS, npc


def _pack_core(core, gi, T_fix, CHUNKS, npc, key, value, query, cutoff, dst):
    g = gi[core]
    pad = g < 0
    gc = np.clip(g, 0, E - 1)
    n = g.size
    qkT = np.empty((2 * 480, n), np.float16)
    qkT[:480] = query[gc].T.astype(np.float16)
    qkT[480:960] = key[gc].T.astype(np.float16)
    v = np.ascontiguousarray(value[gc][:, PERM].astype(np.float16))
    cut = (cutoff[gc] * SCALE).astype(np.float32)
    cut[pad] = 0.0
    chunk_of = np.repeat(np.arange(CHUNKS), T_fix * P)
    dstrel = (dst[gc] - (core * npc + chunk_of * P)).astype(np.float32)
    dstrel[pad] = -5.0
    T_tot = CHUNKS * T_fix
    cut2 = np.ascontiguousarray(cut.reshape(T_tot, P).T)
    dstrel2 = np.ascontiguousarray(dstrel.reshape(T_tot, P).T)
    return {"qkT": np.ascontiguousarray(qkT),
            "v": v, "cut": cut2, "dstr": dstrel2, "memb": _memb()}


def _build_program(T_fix, CHUNKS, reps=1):
    import contextlib
    import os

    import concourse.bacc as bacc
    import concourse.mybir as mybir
    import concourse.tile as tile
    from concourse import bass_isa

    probe = os.environ.get("PROBE", "")

    f32 = mybir.dt.float32
    f16 = mybir.dt.float16
    T_tot = CHUNKS * T_fix
    Epc = T_tot * P

    nc = bacc.Bacc("TRN2", target_bir_lowering=False, debug=False,
                   num_devices=NCORES)
    qkT_d = nc.dram_tensor("qkT", [2 * 480, Epc], f16,
                           kind="ExternalInput").ap()
    v_d = nc.dram_tensor("v", [Epc, 480], f16, kind="ExternalInput").ap()
    cut_d = nc.dram_tensor("cut", [P, T_tot], f32, kind="ExternalInput").ap()
    dstr_d = nc.dram_tensor("dstr", [P, T_tot], f32, kind="ExternalInput").ap()
    memb_d = nc.dram_tensor("memb", [480, 8], f16, kind="ExternalInput").ap()
    out_d = nc.dram_tensor("out", [CHUNKS * P, 480], f16,
                           kind="ExternalOutput").ap()

    with tile.TileContext(nc) as tc:
        with (
            tc.tile_pool(name="const", bufs=1) as const_pool,
            tc.tile_pool(name="qk", bufs=4) as qk_pool,
            tc.tile_pool(name="w", bufs=4) as w_pool,
            tc.tile_pool(name="v", bufs=4) as v_pool,
            tc.tile_pool(name="oh", bufs=3) as oh_pool,
            tc.tile_pool(name="stat", bufs=6) as stat_pool,
            tc.tile_pool(name="outp", bufs=3) as out_pool,
            tc.tile_pool(name="psw", bufs=2, space="PSUM") as psw_pool,
            tc.tile_pool(name="pso", bufs=2, space="PSUM") as pso_pool,
            tc.tile_pool(name="psd", bufs=2, space="PSUM") as psd_pool,
            tc.tile_pool(name="statps", bufs=1, space="PSUM") as statps_pool,
        ):
            iota_i = const_pool.tile([P, P], mybir.dt.int32)
            nc.gpsimd.iota(iota_i[:], pattern=[[1, P]], base=0,
                           channel_multiplier=0)
            iota_f = const_pool.tile([P, P], f16)
            nc.vector.tensor_copy(iota_f[:], iota_i[:])
            iotach_i = const_pool.tile([P, T_fix * P], mybir.dt.int32)
            nc.gpsimd.iota(iotach_i[:], pattern=[[0, T_fix], [1, P]], base=0,
                           channel_multiplier=0)
            iotach_f = const_pool.tile([P, T_fix * P], f16)
            nc.vector.tensor_copy(iotach_f[:], iotach_i[:])
            iotac_i = const_pool.tile([P, 1], mybir.dt.int32)
            nc.gpsimd.iota(iotac_i[:], pattern=[[0, 1]], base=0,
                           channel_multiplier=1)
            iotac_f = const_pool.tile([P, 1], f32)
            nc.vector.tensor_copy(iotac_f[:], iotac_i[:])
            ident = const_pool.tile([P, P], f16)
            nc.vector.tensor_scalar(out=ident[:], in0=iota_f[:],
                                    scalar1=iotac_f[:], scalar2=None,
                                    op0=mybir.AluOpType.is_equal)
            ones_row = const_pool.tile([1, P], f16)
            nc.vector.memset(ones_row[:], 1.0)
            cut_sb = const_pool.tile([P, T_tot], f32)
            nc.sync.dma_start(out=cut_sb[:], in_=cut_d[:, :])
            dstr_sb = const_pool.tile([P, T_tot], f32)
            nc.sync.dma_start(out=dstr_sb[:], in_=dstr_d[:, :])
            memb_sb = const_pool.tile([GW, FG * 8], f16)
            for g in range(FG):
                nc.sync.dma_start(out=memb_sb[:, g * 8:(g + 1) * 8],
                                  in_=memb_d[g * GW:(g + 1) * GW, :])

            def chunk_body_dma(c):
                # PROBE=dma: stream the same bytes on the same queues, no
                # compute. Output is garbage; only the timing is meaningful.
                for si, s0 in enumerate(range(0, T_fix, SPAN)):
                    sw = min(SPAN, T_fix - s0)
                    e0 = (c * T_fix + s0) * P
                    ew = sw * P
                    qk = qk_pool.tile([GW, 2 * FG * ew], f16)
                    qk_eng = nc.sync if si % 2 == 0 else nc.scalar
                    qk_eng.dma_start(
                        out=qk[:].rearrange("p (g e) -> p g e", g=2 * FG),
                        in_=qkT_d[:, e0:e0 + ew].rearrange(
                            "(g p) e -> p g e", p=GW))
                    vs = v_pool.tile([P, sw * 480], f16)
                    v_eng = nc.scalar if si % 2 == 0 else nc.sync
                    v_eng.dma_start(
                        out=vs[:].rearrange("p (t f) -> p t f", t=sw),
                        in_=v_d[e0:e0 + ew, :].rearrange("(t p) f -> p t f",
                                                         p=P))
                outt = out_pool.tile([P, 480], f16)
                nc.vector.memset(outt[:], 0.0)
                nc.sync.dma_start(out=out_d[c * P:(c + 1) * P, :], in_=outt[:])

            def chunk_body(c):
                # one-hot matrices for the whole chunk (data-independent,
                # built while the DMAs stream; tensor_scalar runs in DVE 4x
                # mode since the per-partition scalar operand is exempt)
                oh = oh_pool.tile([P, T_fix * P], f16)
                for t in range(T_fix):
                    nc.vector.tensor_scalar(
                        out=oh[:, t * P:(t + 1) * P], in0=iota_f[:],
                        scalar1=dstr_sb[:, c * T_fix + t:c * T_fix + t + 1],
                        scalar2=None, op0=mybir.AluOpType.is_equal)

                w_chunk = w_pool.tile([P, T_fix * 8], f16)
                for si, s0 in enumerate(range(0, T_fix, SPAN)):
                    sw = min(SPAN, T_fix - s0)
                    e0 = (c * T_fix + s0) * P
                    ew = sw * P
                    # one DMA for the span's q and k (all 8 feature groups),
                    # alternating between the SP and ACT hwdge queues
                    qk = qk_pool.tile([GW, 2 * FG * ew], f16)
                    qk_eng = nc.sync if si % 2 == 0 else nc.scalar
                    qk_eng.dma_start(
                        out=qk[:].rearrange("p (g e) -> p g e", g=2 * FG),
                        in_=qkT_d[:, e0:e0 + ew].rearrange(
                            "(g p) e -> p g e", p=GW))
                    # prod[g] = q[g] * k[g], in place over the q half
                    nc.vector.tensor_mul(qk[:, 0:FG * ew], qk[:, 0:FG * ew],
                                         qk[:, FG * ew:2 * FG * ew])
                    for tl in range(sw):
                        t = s0 + tl
                        gidx = c * T_fix + t
                        psw = psw_pool.tile([P, 8], f32)
                        for g in range(FG):
                            nc.tensor.matmul(
                                out=psw[:],
                                lhsT=qk[:, g * ew + tl * P:
                                        g * ew + (tl + 1) * P],
                                rhs=memb_sb[:, g * 8:(g + 1) * 8],
                                start=(g == 0), stop=(g == FG - 1))
                        nc.vector.tensor_scalar(
                            out=w_chunk[:, t * 8:(t + 1) * 8], in0=psw[:],
                            scalar1=cut_sb[:, gidx:gidx + 1], scalar2=None,
                            op0=mybir.AluOpType.mult)

                # chunk max across partitions via PE transpose + broadcast
                # (gpsimd partition_all_reduce is a slow Q7 software op)
                wmax = stat_pool.tile([P, 1], f16)
                nc.vector.reduce_max(out=wmax[:], in_=w_chunk[:],
                                     axis=mybir.AxisListType.X)
                ps_t = statps_pool.tile([1, P], f32)
                nc.tensor.matmul(out=ps_t[:], lhsT=wmax[:], rhs=ident[:],
                                 start=True, stop=True)
                cmax1 = stat_pool.tile([1, 1], f16)
                nc.vector.reduce_max(out=cmax1[:], in_=ps_t[:],
                                     axis=mybir.AxisListType.X)
                ps_b = statps_pool.tile([P, 1], f32)
                nc.tensor.matmul(out=ps_b[:], lhsT=ones_row[:], rhs=cmax1[:],
                                 start=True, stop=True)
                negC = stat_pool.tile([P, 1], f32)
                nc.vector.tensor_scalar_mul(negC[:], ps_b[:], -1.0)
                wexp = w_pool.tile([P, T_fix * 8], f16)
                nc.scalar.activation(wexp[:], w_chunk[:],
                                     mybir.ActivationFunctionType.Exp,
                                     bias=negC[:], scale=1.0)

                pso = pso_pool.tile([P, 480], f32)
                psd = psd_pool.tile([P, 8], f32)
                for si, s0 in enumerate(range(0, T_fix, SPAN)):
                    sw = min(SPAN, T_fix - s0)
                    e0 = (c * T_fix + s0) * P
                    ew = sw * P
                    # one DMA for the span's v tiles, opposite queue to qk
                    vs = v_pool.tile([P, sw * 480], f16)
                    v_eng = nc.scalar if si % 2 == 0 else nc.sync
                    v_eng.dma_start(
                        out=vs[:].rearrange("p (t f) -> p t f", t=sw),
                        in_=v_d[e0:e0 + ew, :].rearrange("(t p) f -> p t f",
                                                         p=P))
                    # one w*v multiply for the span, in place (4D, 2x)
                    nc.vector.tensor_mul(
                        vs[:].rearrange("p (t d h) -> p t d h", t=sw, h=8),
                        vs[:].rearrange("p (t d h) -> p t d h", t=sw, h=8),
                        wexp[:, s0 * 8:(s0 + sw) * 8]
                        .rearrange("p (t h) -> p t h", t=sw)
                        .unsqueeze(2).to_broadcast([P, sw, 60, 8]))
                    for tl in range(sw):
                        t = s0 + tl
                        nc.tensor.matmul(out=pso[:],
                                         lhsT=oh[:, t * P:(t + 1) * P],
                                         rhs=vs[:, tl * 480:(tl + 1) * 480],
                                         start=(t == 0), stop=(t == T_fix - 1))
                        nc.tensor.matmul(out=psd[:],
                                         lhsT=oh[:, t * P:(t + 1) * P],
                                         rhs=wexp[:, t * 8:(t + 1) * 8],
                                         start=(t == 0), stop=(t == T_fix - 1))

                srec = stat_pool.tile([P, 8], f32)
                nc.vector.tensor_scalar_add(srec[:], psd[:], 1e-30)
                nc.vector.reciprocal(srec[:], srec[:])
                outt = out_pool.tile([P, 480], f16)
                nc.vector.tensor_mul(
                    outt[:].rearrange("p (d h) -> p d h", h=8),
                    pso[:].rearrange("p (d h) -> p d h", h=8),
                    srec[:].unsqueeze(1).to_broadcast([P, 60, 8]))
                nc.sync.dma_start(out=out_d[c * P:(c + 1) * P, :], in_=outt[:])

            # reps>1 wraps the body in a hardware loop purely for timing
            body = chunk_body_dma if probe == "dma" else chunk_body
            loop = tc.For_i(0, reps, 1) if reps > 1 else contextlib.nullcontext()
            with loop:
                for c in range(CHUNKS):
                    body(c)

    nc.compile()
    return nc


def _unpermute(packed):
    # packed [-, 480] d-major -> fused layout, f32
    out = np.empty((packed.shape[0], 480), np.float32)
    out[:, PERM] = packed.astype(np.float32)
    return out


def kernel(key, value, query, edge_weight_cutoff, edge_index, num_nodes):
    key = np.ascontiguousarray(np.asarray(key, dtype=np.float32))
    value = np.ascontiguousarray(np.asarray(value, dtype=np.float32))
    query = np.ascontiguousarray(np.asarray(query, dtype=np.float32))
    cutoff = np.asarray(edge_weight_cutoff, dtype=np.float32)
    dst = np.asarray(edge_index)[1].astype(np.int64)

    gi, T_fix, CHUNKS, npc = _plan_shard(dst)
    in_maps = [_pack_core(core, gi, T_fix, CHUNKS, npc,
                          key, value, query, cutoff, dst)
               for core in range(NCORES)]

    nc = _build_program(T_fix, CHUNKS)

    from concourse.bass_utils import run_bass_kernel_spmd
    res = run_bass_kernel_spmd(nc, in_maps, core_ids=list(range(NCORES)))
    out = np.concatenate([_unpermute(r["out"][:npc]) for r in res.results])
    return np.ascontiguousarray(out)


if __name__ == "__main__":
    rng = np.random.default_rng(0)
    inputs = {
        "key": rng.standard_normal((E, D)).astype(np.float32),
        "value": rng.standard_normal((E, D)).astype(np.float32),
        "query": rng.standard_normal((E, D)).astype(np.float32),
        "edge_weight_cutoff": rng.random(E).astype(np.float32),
        "edge_index": rng.integers(0, N, (2, E)),
        "num_nodes": N,
    }
    out = kernel(**inputs)
    print("out", out.shape, out.dtype, float(np.abs(out).max()))


# revision 54
# speedup vs baseline: 1.1276x; 1.1276x over previous
"""Trainium2 Bass kernel for nn_AttentionBlockE3 (segment-softmax GNN attention).

Strategy: host sorts edges by destination node and partitions NODES across the
8 cores (1250 nodes each) so each core owns all edges of its nodes — no
collectives needed. Edges are packed per (core, node-chunk-of-128) into a
fixed budget of T_fix 128-edge tiles (padding edges get cutoff=0 / dst=-5 so
they contribute nothing).

v2: everything fp16 (halves the HBM traffic, which is the bottleneck), and
the per-head q.k reduction is done on the Tensor engine instead of DVE:
q and k are packed feature-transposed ([4*128 feature rows, E]); DVE forms
prod = q*k in one packed fp16 multiply (2x mode), then 4 matmuls against a
constant 0/1 head-membership matrix reduce features -> logits [128 edges, 8]
directly in PSUM. v is packed d-major (col = d*8+h) so the w*v broadcast
multiply keeps the last dim packed (DVE 2x mode); the host un-permutes the
output columns at the end.

Device program per core (all static addressing, shared by all 8 cores):
  pass 1  stream qT/kT spans, prod = qT*kT (DVE 2x), 4 membership matmuls
          -> logits psum [128,8] per tile, scale by cutoff (tensor_scalar)
  chunk   global-max-subtract softmax: exp(w - C) on ACT, C = chunk max
  pass 2  stream v tiles, rhs = wexp*v (DVE 2x d-major), one-hot via
          tensor_scalar(is_equal), matmuls accumulate [128 nodes, 480] + the
          denominator [128, 8] in PSUM
  epilog  out = pso * recip(psd + 1e-30), DMA fp16 to the node slice
"""
import numpy as np

E, D, N, H = 200000, 480, 10000, 8
P = 128
NCORES = 8
SCALE = 1.0 / np.sqrt(60.0)
FG = 4           # feature groups of 128 partitions (480 padded to 512)
SPAN = 11        # edge tiles per DMA/prod-multiply span


def _perm_dmajor():
    # packed col d*8+h  <-  fused col (per-irrep head-major layout)
    perm = np.zeros(480, np.int64)
    for h in range(8):
        for d in range(60):
            if d < 16:
                fused = h * 16 + d
            elif d < 40:
                fused = 128 + h * 24 + (d - 16)
            else:
                fused = 320 + h * 20 + (d - 40)
            perm[d * 8 + h] = fused
    return perm


PERM = _perm_dmajor()


GW = 120         # feature-group width (480 = 4 * 120, no padding)


def _memb():
    # [480, 8] 0/1 head membership of each fused feature row
    m = np.zeros((480, 8), np.float16)
    for f in range(480):
        if f < 128:
            h = f // 16
        elif f < 320:
            h = (f - 128) // 24
        else:
            h = (f - 320) // 20
        m[f, h] = 1.0
    return m


def _plan_shard(dst):
    """Assign 80 fixed 128-node windows to (core, slot) so that every core's
    slot c has the same tile budget T[c] (required: one SPMD program). Sorting
    windows by tile count and grouping 8 per slot minimizes sum(max) padding
    vs a uniform global T_fix."""
    npc = N // NCORES                       # 1250 nodes per core-block
    CHUNKS = (npc + P - 1) // P             # 10 windows per core-block
    order = np.argsort(dst, kind="stable")
    dst_s = dst[order]
    lo = np.array([b * npc + c * P
                   for b in range(NCORES) for c in range(CHUNKS)])
    hi = np.array([b * npc + min((c + 1) * P, npc)
                   for b in range(NCORES) for c in range(CHUNKS)])
    starts = np.searchsorted(dst_s, lo, side="left")
    ends = np.searchsorted(dst_s, hi, side="left")
    counts = ends - starts
    tiles = (counts + P - 1) // P
    rank = np.argsort(-tiles, kind="stable")    # windows by tile count desc
    T = [int(tiles[rank[8 * k]]) for k in range(CHUNKS)]  # slot budgets
    Ttot = sum(T)
    gi = np.full((NCORES, Ttot * P), -1, np.int64)
    dstoff = np.zeros((NCORES, Ttot * P), np.int64)
    wins = np.zeros((NCORES, CHUNKS, 2), np.int64)   # node range per slot
    base = 0
    for k in range(CHUNKS):
        for core in range(NCORES):
            w = rank[8 * k + core]
            n = counts[w]
            gi[core, base:base + n] = order[starts[w]:ends[w]]
            dstoff[core, base:base + n] = lo[w]
            wins[core, k] = (lo[w], hi[w])
        base += T[k] * P
    return {"gi": gi, "dstoff": dstoff, "T": T, "CHUNKS": CHUNKS,
            "wins": wins}


def _pack_core(core, plan, key, value, query, cutoff, dst):
    g = plan["gi"][core]
    pad = g < 0
    gc = np.clip(g, 0, E - 1)
    n = g.size
    qkT = np.empty((2 * 480, n), np.float16)
    qkT[:480] = query[gc].T.astype(np.float16)
    qkT[480:960] = key[gc].T.astype(np.float16)
    v = np.ascontiguousarray(value[gc][:, PERM].astype(np.float16))
    cut = (cutoff[gc] * SCALE).astype(np.float32)
    cut[pad] = 0.0
    dstrel = (dst[gc] - plan["dstoff"][core]).astype(np.float32)
    dstrel[pad] = -5.0
    T_tot = n // P
    cut2 = np.ascontiguousarray(cut.reshape(T_tot, P).T)
    dstrel2 = np.ascontiguousarray(dstrel.reshape(T_tot, P).T)
    return {"qkT": np.ascontiguousarray(qkT),
            "v": v, "cut": cut2, "dstr": dstrel2, "memb": _memb()}


def _build_program(plan, reps=1):
    import contextlib
    import os

    import concourse.bacc as bacc
    import concourse.mybir as mybir
    import concourse.tile as tile
    from concourse import bass_isa

    probe = os.environ.get("PROBE", "")

    f32 = mybir.dt.float32
    f16 = mybir.dt.float16
    T = plan["T"]
    CHUNKS = plan["CHUNKS"]
    tbase = [sum(T[:c]) for c in range(CHUNKS)]   # tile base of each slot
    T_tot = sum(T)
    Epc = T_tot * P

    nc = bacc.Bacc("TRN2", target_bir_lowering=False, debug=False,
                   num_devices=NCORES)
    qkT_d = nc.dram_tensor("qkT", [2 * 480, Epc], f16,
                           kind="ExternalInput").ap()
    v_d = nc.dram_tensor("v", [Epc, 480], f16, kind="ExternalInput").ap()
    cut_d = nc.dram_tensor("cut", [P, T_tot], f32, kind="ExternalInput").ap()
    dstr_d = nc.dram_tensor("dstr", [P, T_tot], f32, kind="ExternalInput").ap()
    memb_d = nc.dram_tensor("memb", [480, 8], f16, kind="ExternalInput").ap()
    out_d = nc.dram_tensor("out", [CHUNKS * P, 480], f16,
                           kind="ExternalOutput").ap()

    with tile.TileContext(nc) as tc:
        with (
            tc.tile_pool(name="const", bufs=1) as const_pool,
            tc.tile_pool(name="qk", bufs=4) as qk_pool,
            tc.tile_pool(name="w", bufs=4) as w_pool,
            tc.tile_pool(name="v", bufs=4) as v_pool,
            tc.tile_pool(name="oh", bufs=3) as oh_pool,
            tc.tile_pool(name="stat", bufs=6) as stat_pool,
            tc.tile_pool(name="outp", bufs=3) as out_pool,
            tc.tile_pool(name="psw", bufs=2, space="PSUM") as psw_pool,
            tc.tile_pool(name="pso", bufs=2, space="PSUM") as pso_pool,
            tc.tile_pool(name="psd", bufs=2, space="PSUM") as psd_pool,
            tc.tile_pool(name="statps", bufs=1, space="PSUM") as statps_pool,
        ):
            iota_i = const_pool.tile([P, P], mybir.dt.int32)
            nc.gpsimd.iota(iota_i[:], pattern=[[1, P]], base=0,
                           channel_multiplier=0)
            iota_f = const_pool.tile([P, P], f16)
            nc.vector.tensor_copy(iota_f[:], iota_i[:])
            iotac_i = const_pool.tile([P, 1], mybir.dt.int32)
            nc.gpsimd.iota(iotac_i[:], pattern=[[0, 1]], base=0,
                           channel_multiplier=1)
            iotac_f = const_pool.tile([P, 1], f32)
            nc.vector.tensor_copy(iotac_f[:], iotac_i[:])
            ident = const_pool.tile([P, P], f16)
            nc.vector.tensor_scalar(out=ident[:], in0=iota_f[:],
                                    scalar1=iotac_f[:], scalar2=None,
                                    op0=mybir.AluOpType.is_equal)
            ones_row = const_pool.tile([1, P], f16)
            nc.vector.memset(ones_row[:], 1.0)
            cut_sb = const_pool.tile([P, T_tot], f32)
            nc.sync.dma_start(out=cut_sb[:], in_=cut_d[:, :])
            dstr_sb = const_pool.tile([P, T_tot], f32)
            nc.sync.dma_start(out=dstr_sb[:], in_=dstr_d[:, :])
            memb_sb = const_pool.tile([GW, FG * 8], f16)
            for g in range(FG):
                nc.sync.dma_start(out=memb_sb[:, g * 8:(g + 1) * 8],
                                  in_=memb_d[g * GW:(g + 1) * GW, :])

            def chunk_body_dma(c):
                # PROBE=dma: stream the same bytes on the same queues, no
                # compute. Output is garbage; only the timing is meaningful.
                Tc = T[c]
                for si, s0 in enumerate(range(0, Tc, SPAN)):
                    sw = min(SPAN, Tc - s0)
                    e0 = (tbase[c] + s0) * P
                    ew = sw * P
                    qk = qk_pool.tile([GW, 2 * FG * ew], f16)
                    qk_eng = nc.sync if si % 2 == 0 else nc.scalar
                    qk_eng.dma_start(
                        out=qk[:].rearrange("p (g e) -> p g e", g=2 * FG),
                        in_=qkT_d[:, e0:e0 + ew].rearrange(
                            "(g p) e -> p g e", p=GW))
                    vs = v_pool.tile([P, sw * 480], f16)
                    v_eng = nc.scalar if si % 2 == 0 else nc.sync
                    v_eng.dma_start(
                        out=vs[:].rearrange("p (t f) -> p t f", t=sw),
                        in_=v_d[e0:e0 + ew, :].rearrange("(t p) f -> p t f",
                                                         p=P))
                outt = out_pool.tile([P, 480], f16)
                nc.vector.memset(outt[:], 0.0)
                nc.sync.dma_start(out=out_d[c * P:(c + 1) * P, :], in_=outt[:])

            def chunk_body(c):
                Tc = T[c]
                tb = tbase[c]
                # one-hot matrices for the whole chunk (data-independent,
                # built while the DMAs stream; tensor_scalar runs in DVE 4x
                # mode since the per-partition scalar operand is exempt)
                oh = oh_pool.tile([P, Tc * P], f16)
                for t in range(Tc):
                    nc.vector.tensor_scalar(
                        out=oh[:, t * P:(t + 1) * P], in0=iota_f[:],
                        scalar1=dstr_sb[:, tb + t:tb + t + 1],
                        scalar2=None, op0=mybir.AluOpType.is_equal)

                w_chunk = w_pool.tile([P, Tc * 8], f16)
                for si, s0 in enumerate(range(0, Tc, SPAN)):
                    sw = min(SPAN, Tc - s0)
                    e0 = (tb + s0) * P
                    ew = sw * P
                    # one DMA for the span's q and k (all 8 feature groups),
                    # alternating between the SP and ACT hwdge queues
                    qk = qk_pool.tile([GW, 2 * FG * ew], f16)
                    qk_eng = nc.sync if si % 2 == 0 else nc.scalar
                    qk_eng.dma_start(
                        out=qk[:].rearrange("p (g e) -> p g e", g=2 * FG),
                        in_=qkT_d[:, e0:e0 + ew].rearrange(
                            "(g p) e -> p g e", p=GW))
                    # prod[g] = q[g] * k[g], in place over the q half
                    nc.vector.tensor_mul(qk[:, 0:FG * ew], qk[:, 0:FG * ew],
                                         qk[:, FG * ew:2 * FG * ew])
                    for tl in range(sw):
                        t = s0 + tl
                        gidx = tb + t
                        psw = psw_pool.tile([P, 8], f32)
                        for g in range(FG):
                            nc.tensor.matmul(
                                out=psw[:],
                                lhsT=qk[:, g * ew + tl * P:
                                        g * ew + (tl + 1) * P],
                                rhs=memb_sb[:, g * 8:(g + 1) * 8],
                                start=(g == 0), stop=(g == FG - 1))
                        nc.vector.tensor_scalar(
                            out=w_chunk[:, t * 8:(t + 1) * 8], in0=psw[:],
                            scalar1=cut_sb[:, gidx:gidx + 1], scalar2=None,
                            op0=mybir.AluOpType.mult)

                # chunk max across partitions via PE transpose + broadcast
                # (gpsimd partition_all_reduce is a slow Q7 software op)
                wmax = stat_pool.tile([P, 1], f16)
                nc.vector.reduce_max(out=wmax[:], in_=w_chunk[:],
                                     axis=mybir.AxisListType.X)
                ps_t = statps_pool.tile([1, P], f32)
                nc.tensor.matmul(out=ps_t[:], lhsT=wmax[:], rhs=ident[:],
                                 start=True, stop=True)
                cmax1 = stat_pool.tile([1, 1], f16)
                nc.vector.reduce_max(out=cmax1[:], in_=ps_t[:],
                                     axis=mybir.AxisListType.X)
                ps_b = statps_pool.tile([P, 1], f32)
                nc.tensor.matmul(out=ps_b[:], lhsT=ones_row[:], rhs=cmax1[:],
                                 start=True, stop=True)
                negC = stat_pool.tile([P, 1], f32)
                nc.vector.tensor_scalar_mul(negC[:], ps_b[:], -1.0)
                wexp = w_pool.tile([P, Tc * 8], f16)
                nc.scalar.activation(wexp[:], w_chunk[:],
                                     mybir.ActivationFunctionType.Exp,
                                     bias=negC[:], scale=1.0)

                pso = pso_pool.tile([P, 480], f32)
                psd = psd_pool.tile([P, 8], f32)
                for si, s0 in enumerate(range(0, Tc, SPAN)):
                    sw = min(SPAN, Tc - s0)
                    e0 = (tb + s0) * P
                    ew = sw * P
                    # one DMA for the span's v tiles, opposite queue to qk
                    vs = v_pool.tile([P, sw * 480], f16)
                    v_eng = nc.scalar if si % 2 == 0 else nc.sync
                    v_eng.dma_start(
                        out=vs[:].rearrange("p (t f) -> p t f", t=sw),
                        in_=v_d[e0:e0 + ew, :].rearrange("(t p) f -> p t f",
                                                         p=P))
                    # one w*v multiply for the span, in place (4D, 2x)
                    nc.vector.tensor_mul(
                        vs[:].rearrange("p (t d h) -> p t d h", t=sw, h=8),
                        vs[:].rearrange("p (t d h) -> p t d h", t=sw, h=8),
                        wexp[:, s0 * 8:(s0 + sw) * 8]
                        .rearrange("p (t h) -> p t h", t=sw)
                        .unsqueeze(2).to_broadcast([P, sw, 60, 8]))
                    for tl in range(sw):
                        t = s0 + tl
                        nc.tensor.matmul(out=pso[:],
                                         lhsT=oh[:, t * P:(t + 1) * P],
                                         rhs=vs[:, tl * 480:(tl + 1) * 480],
                                         start=(t == 0), stop=(t == Tc - 1))
                        nc.tensor.matmul(out=psd[:],
                                         lhsT=oh[:, t * P:(t + 1) * P],
                                         rhs=wexp[:, t * 8:(t + 1) * 8],
                                         start=(t == 0), stop=(t == Tc - 1))

                srec = stat_pool.tile([P, 8], f32)
                nc.vector.tensor_scalar_add(srec[:], psd[:], 1e-30)
                nc.vector.reciprocal(srec[:], srec[:])
                outt = out_pool.tile([P, 480], f16)
                nc.vector.tensor_mul(
                    outt[:].rearrange("p (d h) -> p d h", h=8),
                    pso[:].rearrange("p (d h) -> p d h", h=8),
                    srec[:].unsqueeze(1).to_broadcast([P, 60, 8]))
                nc.sync.dma_start(out=out_d[c * P:(c + 1) * P, :], in_=outt[:])

            # reps>1 wraps the body in a hardware loop purely for timing
            body = chunk_body_dma if probe == "dma" else chunk_body
            loop = tc.For_i(0, reps, 1) if reps > 1 else contextlib.nullcontext()
            with loop:
                for c in range(CHUNKS):
                    body(c)

    nc.compile()
    return nc


def _unpermute(packed):
    # packed [-, 480] d-major -> fused layout, f32
    out = np.empty((packed.shape[0], 480), np.float32)
    out[:, PERM] = packed.astype(np.float32)
    return out


def _assemble(plan, stacked):
    # stacked [NCORES, CHUNKS*P, 480] fp16 d-major, slot-row order -> [N, 480]
    out = np.zeros((N, 480), np.float32)
    for core in range(NCORES):
        for k in range(plan["CHUNKS"]):
            lo, hi = plan["wins"][core, k]
            out[lo:hi] = _unpermute(stacked[core, k * P:k * P + (hi - lo)])
    return out


def kernel(key, value, query, edge_weight_cutoff, edge_index, num_nodes):
    key = np.ascontiguousarray(np.asarray(key, dtype=np.float32))
    value = np.ascontiguousarray(np.asarray(value, dtype=np.float32))
    query = np.ascontiguousarray(np.asarray(query, dtype=np.float32))
    cutoff = np.asarray(edge_weight_cutoff, dtype=np.float32)
    dst = np.asarray(edge_index)[1].astype(np.int64)

    plan = _plan_shard(dst)
    in_maps = [_pack_core(core, plan, key, value, query, cutoff, dst)
               for core in range(NCORES)]

    nc = _build_program(plan)

    from concourse.bass_utils import run_bass_kernel_spmd
    res = run_bass_kernel_spmd(nc, in_maps, core_ids=list(range(NCORES)))
    stacked = np.stack([r["out"] for r in res.results])
    return np.ascontiguousarray(_assemble(plan, stacked))


if __name__ == "__main__":
    rng = np.random.default_rng(0)
    inputs = {
        "key": rng.standard_normal((E, D)).astype(np.float32),
        "value": rng.standard_normal((E, D)).astype(np.float32),
        "query": rng.standard_normal((E, D)).astype(np.float32),
        "edge_weight_cutoff": rng.random(E).astype(np.float32),
        "edge_index": rng.integers(0, N, (2, E)),
        "num_nodes": N,
    }
    out = kernel(**inputs)
    print("out", out.shape, out.dtype, float(np.abs(out).max()))
